# revision 14
# baseline (speedup 1.0000x reference)
"""Trainium2 Bass kernel for nn_Attention_54254026883778.

Single-head attention with an additive post-softmax intensity term:
    q/k/v = X @ W{q,k,v}.T + b;  scores = q k^T / sqrt(D)
    attn  = softmax(scores) + intensity;  out = (attn @ v) @ Wo.T + bo

Sharding: 8 cores = 4 batches x 2 sequence halves. Each core computes
K^T and V for its own 1024 rows; the partner half arrives via 2-rank
AllGathers through shared DRAM (global t-order, SPMD-static).

Precision plan: Q/K projections and the scores matmul run in fp8e4m3
with DoubleRow (2x PE throughput). Host ships X and 32*W{q,k}^T in fp8
(the x32 shift keeps the tiny weights in fp8's normal range); the exp
activation folds the 1/(32*32) back in on top of 1/sqrt(D). Softmax
errors are ~1000x attenuated in the output (softmax rows sum to 1 while
the additive intensity rows sum to ~1024), so fp8 there is safe. The
V projection, PV and output projection stay bf16.

Dataflow per core (own 1024 query rows, full 2048 keys):
    warmup   dummy matmuls under the initial DMA so the PE HAM
             clock-gate opens before real work arrives
    K^T      [dout | t-own]  fp8 DR, j-outer; per-j fp8 AllGather
    V        [t-own | dv]    bf16, dv-half-outer; per-half AllGather
    Q^T      [dout | s]      fp8 DR
    scores   [s | t]         fp8 DR -> exp on ACT with fused
        row-accumulate -> 1/den on DVE -> diag(recip) ->
        attn^T tile = E-slice.T @ diag(recip)  (PE transposes+normalizes)
        -> DVE adds intensity^T while draining PSUM -> attn^T [t | s]
    out^T    [dv | s]    = V-chunk.T @ attn^T   (bf16)
    final^T  [do | s]    = WoT-chunk.T @ out^T  -> DRAM, host transposes
Biases: q/k (x32) and o enter as per-partition adds during PSUM extract;
v enters as a rank-1 fp32r matmul bv (x) (1 + rowsum(I)).

DMA plan (the previous round stalled 50us on queue head-blocking):
  - big inputs are host-swizzled to [P, chunks, cols] so each is ONE
    trigger with 8-16KB contiguous per-partition packets
  - all collective staging/unpacks ride the SWDGE (gpsimd) queue in an
    emission order that never parks an unpack in front of a stage
  - intensity loads + WO + outputs stay on the sync HWDGE queue
"""

import numpy as np
import ml_dtypes

P = 128
D = 1024
S = 2048          # keys per batch (full sequence)
SH = 1024         # query rows owned by each core
DC = D // P       # 8  contraction chunks over model dim
TC = S // P       # 16 t (key) chunks
NT = 512          # matmul moving free dim / psum bank
SJ = SH // NT     # 2  s-tiles of own rows
TJ = S // NT      # 4  t-tiles
SCALE = 1.0 / 32.0        # 1/sqrt(D)
SCALE8 = SCALE / 1024.0   # undo the x32 on q and k

_CACHE = {}


def _build_module():
    import concourse.bass as bass
    import concourse.tile as tile
    import concourse.mybir as mybir
    from concourse import bacc
    from concourse.masks import make_identity

    f32 = mybir.dt.float32
    f32r = mybir.dt.float32r
    bf16 = mybir.dt.bfloat16
    fp8 = mybir.dt.float8e4
    DR = mybir.MatmulPerfMode.DoubleRow
    Exp = mybir.ActivationFunctionType.Exp
    add = mybir.AluOpType.add

    nc = bacc.Bacc("TRN2", target_bir_lowering=False, debug=False,
                   num_devices=8)

    X8_d = nc.dram_tensor("X8T", [P, DC, SH], fp8, kind="ExternalInput")
    XQ_d = nc.dram_tensor("XQT", [P, DC, SH], bf16, kind="ExternalInput")
    WQ_d = nc.dram_tensor("WQ8", [P, DC, D], fp8, kind="ExternalInput")
    WK_d = nc.dram_tensor("WK8", [P, DC, D], fp8, kind="ExternalInput")
    WV_d = nc.dram_tensor("WVT", [P, DC, D], bf16, kind="ExternalInput")
    WO_d = nc.dram_tensor("WOT", [P, DC, D], bf16, kind="ExternalInput")
    BCOL_d = nc.dram_tensor("BCOL", [P, 3 * DC], f32, kind="ExternalInput")
    BROW_d = nc.dram_tensor("BROW", [1, D + SH], f32, kind="ExternalInput")
    IT_d = nc.dram_tensor("IT", [SH, S], bf16, kind="ExternalInput")
    OUT_d = nc.dram_tensor("OUTT", [D, SH], f32, kind="ExternalOutput")

    out_v = OUT_d[:].rearrange("(c p) s -> p c s", p=P)

    GROUPS = [[0, 1], [2, 3], [4, 5], [6, 7]]
    NDR = DC // 2  # 4 contraction pair-chunks for DoubleRow

    with tile.TileContext(nc) as tc:
        with (
            tc.tile_pool(name="persist", bufs=1) as persist,
            tc.tile_pool(name="mm_ps", bufs=6, space="PSUM") as mm_ps,
            tc.tile_pool(name="tr_ps", bufs=2, space="PSUM") as tr_ps,
            tc.tile_pool(name="dram", bufs=1, space="DRAM") as dram_pool,
        ):
            # ---- persistent tiles -------------------------------------
            KT_sb = persist.tile([P, 2, SJ, DC, NT], fp8)   # K^T gathered
            V_sb = persist.tile([P, 2, TC, NT], bf16)       # V [t | half,dv]
            QT_sb = persist.tile([P, DC, SH], fp8, tag="qt_ot")
            X8_sb = persist.tile([P, DC, SH], fp8)          # X^T fp8
            XQ_sb = persist.tile([P, DC, SH], bf16, tag="xq_at")
            KL_sb = persist.tile([P, SJ, DC, NT], fp8)      # K^T local (x32)
            VL_sb = persist.tile([P, SJ, DC, NT], bf16, tag="vl_wo")
            AT_sb = persist.tile([P, TC, SH], bf16, tag="xq_at")  # attn^T
            ACC_sb = persist.tile([P, DC, TJ], f32)
            IT_sb = persist.tile([P, DC, S], bf16)          # intensity^T

            ident = persist.tile([P, P], bf16)
            make_identity(nc, ident)
            WARM_sb = persist.tile([P, NT], bf16)
            nc.vector.memset(WARM_sb[:], 0.0)
            # bq*32|bk*32|bo as per-partition columns, added on PSUM extract
            BCOL_sb = persist.tile([P, 3 * DC], f32)
            nc.sync.dma_start(BCOL_sb[:], BCOL_d[:])
            # bv and the attn rowsums feed the PV rank-1 bias matmul; fp32r
            # operands must come from a rounding instruction, so stage the
            # DMA through a DVE copy.
            BROW_r = persist.tile([1, D + SH], f32r)
            with tc.tile_pool(name="brow_pool", bufs=1) as brow_pool:
                BROW_ld = brow_pool.tile([1, D + SH], f32)
                nc.sync.dma_start(BROW_ld[:], BROW_d[:])
                nc.vector.tensor_copy(out=BROW_r[:], in_=BROW_ld[:])
            BV_sb = BROW_r[0:1, 0:D]
            RS_sb = BROW_r[0:1, D:D + SH]                   # 1 + rowsum(I)

            # ---- PE warmup: dummy matmuls under the initial DMA -------
            for _ in range(8):
                wps = mm_ps.tile([P, NT], f32, tag="mm", name="warm")
                nc.tensor.matmul(wps[:], ident[:], WARM_sb[:],
                                 start=True, stop=True)

            # collective DRAM tiles
            k_in = dram_pool.tile([P, SJ, DC, NT], fp8, name="k_in")
            k_out = dram_pool.tile([2, P, SJ, DC, NT], fp8, name="k_out")
            v_in = [dram_pool.tile([P, DC, NT], bf16, name=f"v_in{h}")
                    for h in range(2)]
            v_out = [dram_pool.tile([2, P, DC, NT], bf16, name=f"v_out{h}")
                     for h in range(2)]

            # ---- phase A: K-local/gather, V-local/gather, Q -----------
            with (
                tc.tile_pool(name="w8pool", bufs=2) as w8pool,
                tc.tile_pool(name="wvpool", bufs=1) as wvpool,
            ):
                # Inputs split across two HWDGE queues (sync + scalar) so
                # the K-path loads (X8 on sync, WK8 on scalar) drain in
                # parallel and K matmuls start ~8us in. IT is sequenced
                # after the sync-side inputs (needed only from ~70us).
                WK_sb = w8pool.tile([P, DC, D], fp8, tag="w8")
                nc.sync.dma_start(X8_sb[:], X8_d[:])
                nc.scalar.dma_start(WK_sb[:], WK_d[:])
                WV_sb = wvpool.tile([P, DC, D], bf16)
                nc.scalar.dma_start(XQ_sb[:], XQ_d[:])
                nc.sync.dma_start(WV_sb[:], WV_d[:])
                WQ_sb = w8pool.tile([P, DC, D], fp8, tag="w8")
                nc.scalar.dma_start(WQ_sb[:], WQ_d[:])
                nc.sync.dma_start(
                    IT_sb[:], IT_d[:].rearrange("(si p) f -> p si f", p=P))

                # K^T local [dout, t-own] fp8 DoubleRow
                for j in range(SJ):
                    for c in range(DC):
                        ps = mm_ps.tile([P, NT], f32, tag="mm", name="ps")
                        for i in range(NDR):
                            nc.tensor.matmul(
                                ps[:],
                                WK_sb[:, 2 * i:2 * i + 2, c * P:(c + 1) * P],
                                X8_sb[:, 2 * i:2 * i + 2,
                                      j * NT:(j + 1) * NT],
                                start=(i == 0), stop=(i == NDR - 1),
                                perf_mode=DR,
                            )
                        nc.vector.tensor_scalar_add(
                            KL_sb[:, j, c, :], ps[:],
                            BCOL_sb[:, DC + c:DC + c + 1])
                # one fp8 K AllGather (1MB in, 2MB out), then unpack right
                # away on the same SWDGE queue: scores are the next consumer
                # and nothing else needs the queue before the V stages.
                nc.gpsimd.dma_start(k_in[:], KL_sb[:])
                nc.gpsimd.collective_compute(
                    "AllGather", mybir.AluOpType.bypass,
                    replica_groups=GROUPS,
                    ins=[k_in.opt()], outs=[k_out.opt()])
                nc.gpsimd.dma_start(KT_sb[:, 0], k_out[0])
                nc.gpsimd.dma_start(KT_sb[:, 1], k_out[1])

                # V local, dv-half-outer (h = output half); VL is [P,h,t,dv]
                # so each half stages as one contiguous 8KB/partition DMA.
                for h in range(2):
                    for t in range(DC):
                        ps = mm_ps.tile([P, NT], f32, tag="mm", name="ps")
                        for dc in range(DC):
                            nc.tensor.matmul(
                                ps[:],
                                XQ_sb[:, dc, t * P:(t + 1) * P],
                                WV_sb[:, dc, h * NT:(h + 1) * NT],
                                start=(dc == 0),
                                stop=(dc == DC - 1),
                            )
                        nc.vector.tensor_copy(
                            out=VL_sb[:, h, t, :], in_=ps[:])
                    nc.gpsimd.dma_start(v_in[h][:], VL_sb[:, h])
                    nc.gpsimd.collective_compute(
                        "AllGather", mybir.AluOpType.bypass,
                        replica_groups=GROUPS,
                        ins=[v_in[h].opt()], outs=[v_out[h].opt()])
                # V unpacks last on the SWDGE queue (PV needs them latest)
                for h in range(2):
                    for b in range(2):
                        nc.gpsimd.dma_start(
                            V_sb[:, h, b * DC:(b + 1) * DC, :], v_out[h][b])

                # Q^T [dout, s-own] fp8 DoubleRow
                for c in range(DC):
                    psl = [mm_ps.tile([P, NT], f32, tag="mm", name="ps")
                           for _ in range(SJ)]
                    for i in range(NDR):
                        for j in range(SJ):
                            nc.tensor.matmul(
                                psl[j][:],
                                WQ_sb[:, 2 * i:2 * i + 2, c * P:(c + 1) * P],
                                X8_sb[:, 2 * i:2 * i + 2,
                                      j * NT:(j + 1) * NT],
                                start=(i == 0), stop=(i == NDR - 1),
                                perf_mode=DR,
                            )
                    for j in range(SJ):
                        nc.vector.tensor_scalar_add(
                            QT_sb[:, c, j * NT:(j + 1) * NT], psl[j][:],
                            BCOL_sb[:, c:c + 1])

            # ---- phase B: scores -> softmax -> +I^T -> attn^T ---------
            # The per-si epilogue (exp -> den -> diag -> transposes -> +I^T)
            # is software-pipelined one si behind the scores matmuls so the
            # PE never waits on the ACT/DVE chain. Transposes are batched 4
            # per PSUM bank so each intensity-add is one 512-wide DVE op.
            with (
                tc.tile_pool(name="e_pool", bufs=2) as e_pool,
                tc.tile_pool(name="stat_pool", bufs=2) as stat_pool,
            ):
                def si_scores(si):
                    E_sb = e_pool.tile([P, S], bf16, tag="e")
                    psl = [mm_ps.tile([P, NT], f32, tag="mm", name="ps")
                           for _ in range(TJ)]
                    for i in range(NDR):
                        for u in range(TJ):
                            nc.tensor.matmul(
                                psl[u][:],
                                QT_sb[:, 2 * i:2 * i + 2,
                                      si * P:(si + 1) * P],
                                KT_sb[:, u // 2, u % 2, 2 * i:2 * i + 2, :],
                                start=(i == 0), stop=(i == NDR - 1),
                                perf_mode=DR,
                            )
                    for u in range(TJ):
                        nc.scalar.activation(
                            E_sb[:, u * NT:(u + 1) * NT], psl[u][:], Exp,
                            scale=SCALE8, accum_out=ACC_sb[:, si, u:u + 1],
                        )
                    den = stat_pool.tile([P, 1], f32, tag="den")
                    recip = stat_pool.tile([P, 1], f32, tag="recip")
                    diag = stat_pool.tile([P, P], bf16, tag="diag")
                    nc.vector.reduce_sum(
                        den[:], ACC_sb[:, si, :], axis=mybir.AxisListType.X)
                    nc.vector.reciprocal(recip[:], den[:])
                    # diag(recip): identity rows scaled per-partition
                    nc.vector.tensor_scalar_mul(diag[:], ident[:], recip[:])
                    return E_sb, diag

                def si_transposes(si, E_sb, diag):
                    # attn^T = E-slice.T @ diag (transpose + normalize), 4
                    # tiles per PSUM bank; PSUM drain adds intensity^T.
                    for t4 in range(TJ):
                        pt = tr_ps.tile([P, NT], f32, tag="tr")
                        for q in range(4):
                            t = t4 * 4 + q
                            nc.tensor.matmul(
                                pt[:, q * P:(q + 1) * P],
                                E_sb[:, t * P:(t + 1) * P], diag[:],
                                start=True, stop=True,
                                skip_group_check=True)
                        nc.vector.tensor_tensor(
                            AT_sb[:, t4 * 4:(t4 + 1) * 4,
                                  si * P:(si + 1) * P],
                            pt[:], IT_sb[:, si, t4 * NT:(t4 + 1) * NT], add)

                prev = None
                for si in range(DC):
                    cur = si_scores(si)
                    if prev is not None:
                        si_transposes(si - 1, *prev)
                    prev = cur
                si_transposes(DC - 1, *prev)

            # ---- phase D/E: PV -> out^T, then projection per s-tile ---
            OT_sb = persist.tile([P, DC, SH], bf16, tag="qt_ot")
            with (
                tc.tile_pool(name="wo_pool", bufs=1) as wo_pool,
                tc.tile_pool(name="fin_pool", bufs=3) as fin_pool,
            ):
                WO_sb = wo_pool.tile([P, DC, D], bf16)
                nc.sync.dma_start(WO_sb[:], WO_d[:])
                for sj in range(SJ):
                    for dvi in range(DC):
                        ps = mm_ps.tile([P, NT], f32, tag="mm")
                        for t in range(TC):
                            nc.tensor.matmul(
                                ps[:],
                                V_sb[:, dvi // 4, t,
                                     (dvi % 4) * P:(dvi % 4 + 1) * P],
                                AT_sb[:, t, sj * NT:(sj + 1) * NT],
                                start=(t == 0),
                                stop=False,
                            )
                        # bias: bv (x) (1 + rowsum(I))
                        nc.tensor.matmul(
                            ps[:], BV_sb[0:1, dvi * P:(dvi + 1) * P],
                            RS_sb[0:1, sj * NT:(sj + 1) * NT],
                            start=False, stop=True)
                        nc.vector.tensor_copy(
                            out=OT_sb[:, dvi, sj * NT:(sj + 1) * NT], in_=ps[:])

                    for doi in range(DC):
                        ps = mm_ps.tile([P, NT], f32, tag="mm")
                        for dvc in range(DC):
                            nc.tensor.matmul(
                                ps[:],
                                WO_sb[:, dvc, doi * P:(doi + 1) * P],
                                OT_sb[:, dvc, sj * NT:(sj + 1) * NT],
                                start=(dvc == 0), stop=(dvc == DC - 1),
                            )
                        F_sb = fin_pool.tile([P, NT], f32, tag="fin")
                        nc.vector.tensor_scalar_add(
                            F_sb[:], ps[:],
                            BCOL_sb[:, 2 * DC + doi:2 * DC + doi + 1])
                        nc.sync.dma_start(
                            out_v[:, doi, sj * NT:(sj + 1) * NT], F_sb[:])

    nc.compile()
    return nc


def _get_module():
    if "nc" not in _CACHE:
        _CACHE["nc"] = _build_module()
    return _CACHE["nc"]


def _chunked(a, ncols):
    """[D, ncols] -> [P, DC, ncols] with partition p holding rows c*128+p."""
    return np.ascontiguousarray(
        a.reshape(DC, P, ncols).transpose(1, 0, 2))


def _make_in_maps(inputs):
    X = np.asarray(inputs["X"], dtype=np.float32)
    intensity = np.asarray(inputs["intensity"], dtype=np.float32)
    bf = ml_dtypes.bfloat16
    f8 = ml_dtypes.float8_e4m3
    Wq8 = _chunked(np.ascontiguousarray(
        np.asarray(inputs["Wq"], np.float32).T * 32.0), D).astype(f8)
    Wk8 = _chunked(np.ascontiguousarray(
        np.asarray(inputs["Wk"], np.float32).T * 32.0), D).astype(f8)
    WvT = _chunked(np.ascontiguousarray(
        np.asarray(inputs["Wv"], np.float32).T), D).astype(bf)
    WoT = _chunked(np.ascontiguousarray(
        np.asarray(inputs["Wo"], np.float32).T), D).astype(bf)
    bq, bk, bv, bo = (np.asarray(inputs[k], np.float32).reshape(D)
                      for k in ("bq", "bk", "bv", "bo"))
    BCOL = np.concatenate(
        [b.reshape(DC, P).T for b in (32.0 * bq, 32.0 * bk, bo)], axis=1
    ).astype(np.float32)  # [128, 24]

    in_maps = []
    for c in range(8):
        b, h = c // 2, c % 2
        XT = np.ascontiguousarray(X[b, h * SH:(h + 1) * SH, :].T)
        XQT = _chunked(XT, SH).astype(bf)
        X8T = _chunked(XT, SH).astype(f8)
        Islc = intensity[b, h * SH:(h + 1) * SH, :]
        # [t, s] -> [si*128+tp, tc*128+sp] so each per-si load is one
        # contiguous row-block (128 descriptors instead of 2048)
        IT = np.ascontiguousarray(
            Islc.T.reshape(TC, P, DC, P).transpose(2, 1, 0, 3).reshape(SH, S)
        ).astype(bf)
        rows = 1.0 + Islc.sum(axis=1, dtype=np.float64).astype(np.float32)
        BROW = np.concatenate([bv, rows]).reshape(1, D + SH)
        in_maps.append({
            "X8T": X8T, "XQT": XQT, "WQ8": Wq8, "WK8": Wk8,
            "WVT": WvT, "WOT": WoT,
            "BCOL": BCOL, "BROW": BROW, "IT": IT,
        })
    return in_maps


def _gather(results):
    out = np.empty((4, S, D), dtype=np.float32)
    for c in range(8):
        b, h = c // 2, c % 2
        out[b, h * SH:(h + 1) * SH, :] = results[c]["OUTT"].T
    return out


def kernel(**inputs):
    from concourse import bass_utils

    in_maps = _make_in_maps(inputs)
    nc = _get_module()
    res = bass_utils.run_bass_kernel_spmd(nc, in_maps, core_ids=list(range(8)))
    return _gather(res.results)


# revision 17
# speedup vs baseline: 1.0128x; 1.0128x over previous
"""Trainium2 Bass kernel for nn_Attention_54254026883778.

Single-head attention with an additive post-softmax intensity term:
    q/k/v = X @ W{q,k,v}.T + b;  scores = q k^T / sqrt(D)
    attn  = softmax(scores) + intensity;  out = (attn @ v) @ Wo.T + bo

Sharding: 8 cores = 4 batches x 2 sequence halves. Each core computes
K^T and V for its own 1024 rows; the partner half arrives via 2-rank
AllGathers through shared DRAM (global t-order, SPMD-static).

Precision plan: Q/K projections and the scores matmul run in fp8e4m3
with DoubleRow (2x PE throughput). Host ships X and 32*W{q,k}^T in fp8
(the x32 shift keeps the tiny weights in fp8's normal range); the exp
activation folds the 1/(32*32) back in on top of 1/sqrt(D). Softmax
errors are ~1000x attenuated in the output (softmax rows sum to 1 while
the additive intensity rows sum to ~1024), so fp8 there is safe. The
V projection, PV and output projection stay bf16.

Dataflow per core (own 1024 query rows, full 2048 keys):
    warmup   dummy matmuls under the initial DMA so the PE HAM
             clock-gate opens before real work arrives
    K^T      [dout | t-own]  fp8 DR, j-outer; per-j fp8 AllGather
    V        [t-own | dv]    bf16, dv-half-outer; per-half AllGather
    Q^T      [dout | s]      fp8 DR
    scores   [s | t]         fp8 DR -> exp on ACT with fused
        row-accumulate -> 1/den on DVE -> diag(recip) ->
        attn^T tile = E-slice.T @ diag(recip)  (PE transposes+normalizes)
        -> DVE adds intensity^T while draining PSUM -> attn^T [t | s]
    out^T    [dv | s]    = V-chunk.T @ attn^T   (bf16)
    final^T  [do | s]    = WoT-chunk.T @ out^T  -> DRAM, host transposes
Biases: q/k (x32) and o enter as per-partition adds during PSUM extract;
v enters as a rank-1 fp32r matmul bv (x) (1 + rowsum(I)).

DMA plan (the previous round stalled 50us on queue head-blocking):
  - big inputs are host-swizzled to [P, chunks, cols] so each is ONE
    trigger with 8-16KB contiguous per-partition packets
  - all collective staging/unpacks ride the SWDGE (gpsimd) queue in an
    emission order that never parks an unpack in front of a stage
  - intensity loads + WO + outputs stay on the sync HWDGE queue
"""

import numpy as np
import ml_dtypes

P = 128
D = 1024
S = 2048          # keys per batch (full sequence)
SH = 1024         # query rows owned by each core
DC = D // P       # 8  contraction chunks over model dim
TC = S // P       # 16 t (key) chunks
NT = 512          # matmul moving free dim / psum bank
SJ = SH // NT     # 2  s-tiles of own rows
TJ = S // NT      # 4  t-tiles
SCALE = 1.0 / 32.0        # 1/sqrt(D)
SCALE8 = SCALE / 1024.0   # undo the x32 on q and k

_CACHE = {}


def _build_module():
    import concourse.bass as bass
    import concourse.tile as tile
    import concourse.mybir as mybir
    from concourse import bacc
    from concourse.masks import make_identity

    f32 = mybir.dt.float32
    f32r = mybir.dt.float32r
    bf16 = mybir.dt.bfloat16
    fp8 = mybir.dt.float8e4
    DR = mybir.MatmulPerfMode.DoubleRow
    Exp = mybir.ActivationFunctionType.Exp
    add = mybir.AluOpType.add

    nc = bacc.Bacc("TRN2", target_bir_lowering=False, debug=False,
                   num_devices=8)

    X8_d = nc.dram_tensor("X8T", [P, DC, SH], fp8, kind="ExternalInput")
    XQ_d = nc.dram_tensor("XQT", [P, DC, SH], bf16, kind="ExternalInput")
    WQ_d = nc.dram_tensor("WQ8", [P, DC, D], fp8, kind="ExternalInput")
    WK_d = nc.dram_tensor("WK8", [P, DC, D], fp8, kind="ExternalInput")
    WV_d = nc.dram_tensor("WVT", [P, DC, D], bf16, kind="ExternalInput")
    WO_d = nc.dram_tensor("WOT", [P, DC, D], bf16, kind="ExternalInput")
    BCOL_d = nc.dram_tensor("BCOL", [P, 3 * DC], f32, kind="ExternalInput")
    BROW_d = nc.dram_tensor("BROW", [1, D + SH], f32, kind="ExternalInput")
    IT_d = nc.dram_tensor("IT", [SH, S], bf16, kind="ExternalInput")
    OUT_d = nc.dram_tensor("OUTT", [D, SH], f32, kind="ExternalOutput")

    out_v = OUT_d[:].rearrange("(c p) s -> p c s", p=P)

    GROUPS = [[0, 1], [2, 3], [4, 5], [6, 7]]
    NDR = DC // 2  # 4 contraction pair-chunks for DoubleRow

    with tile.TileContext(nc) as tc:
        with (
            tc.tile_pool(name="persist", bufs=1) as persist,
            tc.tile_pool(name="mm_ps", bufs=6, space="PSUM") as mm_ps,
            tc.tile_pool(name="tr_ps", bufs=2, space="PSUM") as tr_ps,
            tc.tile_pool(name="dram", bufs=1, space="DRAM") as dram_pool,
        ):
            # ---- persistent tiles -------------------------------------
            KT_sb = persist.tile([P, 2, SJ, DC, NT], fp8)   # K^T gathered
            V_sb = persist.tile([P, 2, TC, NT], bf16)       # V [t | half,dv]
            QT_sb = persist.tile([P, DC, SH], fp8, tag="qt_ot")
            X8_sb = persist.tile([P, DC, SH], fp8)          # X^T fp8
            XQ_sb = persist.tile([P, DC, SH], bf16, tag="xq_at")
            KL_sb = persist.tile([P, SJ, DC, NT], fp8)      # K^T local (x32)
            VL_sb = persist.tile([P, SJ, DC, NT], bf16, tag="vl_wo")
            AT_sb = persist.tile([P, TC, SH], bf16, tag="xq_at")  # attn^T
            ACC_sb = persist.tile([P, DC, TJ], f32)
            IT_sb = persist.tile([P, DC, S], bf16)          # intensity^T

            ident = persist.tile([P, P], bf16)
            make_identity(nc, ident)
            WARM_sb = persist.tile([P, NT], bf16)
            nc.vector.memset(WARM_sb[:], 0.0)
            # bq*32|bk*32|bo as per-partition columns, added on PSUM extract
            # (small/odd-shaped loads ride the SWDGE queue, which is idle
            # until the K staging ~30us in)
            BCOL_sb = persist.tile([P, 3 * DC], f32)
            nc.gpsimd.dma_start(BCOL_sb[:], BCOL_d[:])
            # bv and the attn rowsums feed the PV rank-1 bias matmul; fp32r
            # operands must come from a rounding instruction, so stage the
            # DMA through a DVE copy.
            BROW_r = persist.tile([1, D + SH], f32r)
            with tc.tile_pool(name="brow_pool", bufs=1) as brow_pool:
                BROW_ld = brow_pool.tile([1, D + SH], f32)
                nc.gpsimd.dma_start(BROW_ld[:], BROW_d[:])
                nc.vector.tensor_copy(out=BROW_r[:], in_=BROW_ld[:])
            BV_sb = BROW_r[0:1, 0:D]
            RS_sb = BROW_r[0:1, D:D + SH]                   # 1 + rowsum(I)

            # ---- PE warmup: dummy matmuls under the initial DMA -------
            for _ in range(8):
                wps = mm_ps.tile([P, NT], f32, tag="mm", name="warm")
                nc.tensor.matmul(wps[:], ident[:], WARM_sb[:],
                                 start=True, stop=True)

            # collective DRAM tiles
            k_in = dram_pool.tile([P, SJ, DC, NT], fp8, name="k_in")
            k_out = dram_pool.tile([2, P, SJ, DC, NT], fp8, name="k_out")
            v_in = [dram_pool.tile([P, DC, NT], bf16, name=f"v_in{h}")
                    for h in range(2)]
            v_out = [dram_pool.tile([2, P, DC, NT], bf16, name=f"v_out{h}")
                     for h in range(2)]

            # ---- phase A: K-local/gather, V-local/gather, Q -----------
            with (
                tc.tile_pool(name="w8pool", bufs=2) as w8pool,
                tc.tile_pool(name="wvpool", bufs=1) as wvpool,
            ):
                # Inputs split across three DMA queues so the K-path loads
                # drain in parallel: WK8 on the fast idle SWDGE, X8 at the
                # head of sync, XQ/WQ8 on scalar. IT is sequenced after the
                # sync-side inputs (needed only from ~70us).
                WK_sb = w8pool.tile([P, DC, D], fp8, tag="w8")
                nc.sync.dma_start(X8_sb[:], X8_d[:])
                nc.gpsimd.dma_start(WK_sb[:], WK_d[:])
                WV_sb = wvpool.tile([P, DC, D], bf16)
                nc.scalar.dma_start(XQ_sb[:], XQ_d[:])
                nc.sync.dma_start(WV_sb[:], WV_d[:])
                WQ_sb = w8pool.tile([P, DC, D], fp8, tag="w8")
                nc.scalar.dma_start(WQ_sb[:], WQ_d[:])
                nc.sync.dma_start(
                    IT_sb[:], IT_d[:].rearrange("(si p) f -> p si f", p=P))

                # K^T local [dout, t-own] fp8 DoubleRow
                for j in range(SJ):
                    for c in range(DC):
                        ps = mm_ps.tile([P, NT], f32, tag="mm", name="ps")
                        for i in range(NDR):
                            nc.tensor.matmul(
                                ps[:],
                                WK_sb[:, 2 * i:2 * i + 2, c * P:(c + 1) * P],
                                X8_sb[:, 2 * i:2 * i + 2,
                                      j * NT:(j + 1) * NT],
                                start=(i == 0), stop=(i == NDR - 1),
                                perf_mode=DR,
                            )
                        nc.vector.tensor_scalar_add(
                            KL_sb[:, j, c, :], ps[:],
                            BCOL_sb[:, DC + c:DC + c + 1])
                # one fp8 K AllGather (1MB in, 2MB out), then unpack right
                # away on the same SWDGE queue: scores are the next consumer
                # and nothing else needs the queue before the V stages.
                nc.gpsimd.dma_start(k_in[:], KL_sb[:])
                nc.gpsimd.collective_compute(
                    "AllGather", mybir.AluOpType.bypass,
                    replica_groups=GROUPS,
                    ins=[k_in.opt()], outs=[k_out.opt()])
                nc.gpsimd.dma_start(KT_sb[:, 0], k_out[0])
                nc.gpsimd.dma_start(KT_sb[:, 1], k_out[1])

                # V local, dv-half-outer (h = output half); VL is [P,h,t,dv]
                # so each half stages as one contiguous 8KB/partition DMA.
                for h in range(2):
                    for t in range(DC):
                        ps = mm_ps.tile([P, NT], f32, tag="mm", name="ps")
                        for dc in range(DC):
                            nc.tensor.matmul(
                                ps[:],
                                XQ_sb[:, dc, t * P:(t + 1) * P],
                                WV_sb[:, dc, h * NT:(h + 1) * NT],
                                start=(dc == 0),
                                stop=(dc == DC - 1),
                            )
                        nc.vector.tensor_copy(
                            out=VL_sb[:, h, t, :], in_=ps[:])
                    nc.gpsimd.dma_start(v_in[h][:], VL_sb[:, h])
                    nc.gpsimd.collective_compute(
                        "AllGather", mybir.AluOpType.bypass,
                        replica_groups=GROUPS,
                        ins=[v_in[h].opt()], outs=[v_out[h].opt()])
                # V unpacks last on the SWDGE queue (PV needs them latest)
                for h in range(2):
                    for b in range(2):
                        nc.gpsimd.dma_start(
                            V_sb[:, h, b * DC:(b + 1) * DC, :], v_out[h][b])

                # Q^T [dout, s-own] fp8 DoubleRow
                for c in range(DC):
                    psl = [mm_ps.tile([P, NT], f32, tag="mm", name="ps")
                           for _ in range(SJ)]
                    for i in range(NDR):
                        for j in range(SJ):
                            nc.tensor.matmul(
                                psl[j][:],
                                WQ_sb[:, 2 * i:2 * i + 2, c * P:(c + 1) * P],
                                X8_sb[:, 2 * i:2 * i + 2,
                                      j * NT:(j + 1) * NT],
                                start=(i == 0), stop=(i == NDR - 1),
                                perf_mode=DR,
                            )
                    for j in range(SJ):
                        nc.vector.tensor_scalar_add(
                            QT_sb[:, c, j * NT:(j + 1) * NT], psl[j][:],
                            BCOL_sb[:, c:c + 1])

            # ---- phase B: scores -> softmax -> +I^T -> attn^T ---------
            # The per-si epilogue (exp -> den -> diag -> transposes -> +I^T)
            # is software-pipelined one si behind the scores matmuls so the
            # PE never waits on the ACT/DVE chain. Transposes are batched 4
            # per PSUM bank so each intensity-add is one 512-wide DVE op.
            with (
                tc.tile_pool(name="e_pool", bufs=2) as e_pool,
                tc.tile_pool(name="stat_pool", bufs=2) as stat_pool,
            ):
                def si_scores(si):
                    E_sb = e_pool.tile([P, S], bf16, tag="e")
                    psl = [mm_ps.tile([P, NT], f32, tag="mm", name="ps")
                           for _ in range(TJ)]
                    for i in range(NDR):
                        for u in range(TJ):
                            nc.tensor.matmul(
                                psl[u][:],
                                QT_sb[:, 2 * i:2 * i + 2,
                                      si * P:(si + 1) * P],
                                KT_sb[:, u // 2, u % 2, 2 * i:2 * i + 2, :],
                                start=(i == 0), stop=(i == NDR - 1),
                                perf_mode=DR,
                            )
                    for u in range(TJ):
                        nc.scalar.activation(
                            E_sb[:, u * NT:(u + 1) * NT], psl[u][:], Exp,
                            scale=SCALE8, accum_out=ACC_sb[:, si, u:u + 1],
                        )
                    den = stat_pool.tile([P, 1], f32, tag="den")
                    recip = stat_pool.tile([P, 1], f32, tag="recip")
                    diag = stat_pool.tile([P, P], bf16, tag="diag")
                    nc.vector.reduce_sum(
                        den[:], ACC_sb[:, si, :], axis=mybir.AxisListType.X)
                    nc.vector.reciprocal(recip[:], den[:])
                    # diag(recip): identity rows scaled per-partition
                    nc.vector.tensor_scalar_mul(diag[:], ident[:], recip[:])
                    return E_sb, diag

                def si_transposes(si, E_sb, diag):
                    # attn^T = E-slice.T @ diag (transpose + normalize), 4
                    # tiles per PSUM bank; PSUM drain adds intensity^T.
                    for t4 in range(TJ):
                        pt = tr_ps.tile([P, NT], f32, tag="tr")
                        for q in range(4):
                            t = t4 * 4 + q
                            nc.tensor.matmul(
                                pt[:, q * P:(q + 1) * P],
                                E_sb[:, t * P:(t + 1) * P], diag[:],
                                start=True, stop=True,
                                skip_group_check=True)
                        nc.vector.tensor_tensor(
                            AT_sb[:, t4 * 4:(t4 + 1) * 4,
                                  si * P:(si + 1) * P],
                            pt[:], IT_sb[:, si, t4 * NT:(t4 + 1) * NT], add)

                prev = None
                for si in range(DC):
                    cur = si_scores(si)
                    if prev is not None:
                        si_transposes(si - 1, *prev)
                    prev = cur
                si_transposes(DC - 1, *prev)

            # ---- phase D/E: PV -> out^T, then projection per s-tile ---
            OT_sb = persist.tile([P, DC, SH], bf16, tag="qt_ot")
            with (
                tc.tile_pool(name="wo_pool", bufs=1) as wo_pool,
                tc.tile_pool(name="fin_pool", bufs=3) as fin_pool,
            ):
                WO_sb = wo_pool.tile([P, DC, D], bf16)
                nc.sync.dma_start(WO_sb[:], WO_d[:])
                for sj in range(SJ):
                    for dvi in range(DC):
                        ps = mm_ps.tile([P, NT], f32, tag="mm")
                        for t in range(TC):
                            nc.tensor.matmul(
                                ps[:],
                                V_sb[:, dvi // 4, t,
                                     (dvi % 4) * P:(dvi % 4 + 1) * P],
                                AT_sb[:, t, sj * NT:(sj + 1) * NT],
                                start=(t == 0),
                                stop=False,
                            )
                        # bias: bv (x) (1 + rowsum(I))
                        nc.tensor.matmul(
                            ps[:], BV_sb[0:1, dvi * P:(dvi + 1) * P],
                            RS_sb[0:1, sj * NT:(sj + 1) * NT],
                            start=False, stop=True)
                        nc.vector.tensor_copy(
                            out=OT_sb[:, dvi, sj * NT:(sj + 1) * NT], in_=ps[:])

                    for doi in range(DC):
                        ps = mm_ps.tile([P, NT], f32, tag="mm")
                        for dvc in range(DC):
                            nc.tensor.matmul(
                                ps[:],
                                WO_sb[:, dvc, doi * P:(doi + 1) * P],
                                OT_sb[:, dvc, sj * NT:(sj + 1) * NT],
                                start=(dvc == 0), stop=(dvc == DC - 1),
                            )
                        F_sb = fin_pool.tile([P, NT], f32, tag="fin")
                        nc.vector.tensor_scalar_add(
                            F_sb[:], ps[:],
                            BCOL_sb[:, 2 * DC + doi:2 * DC + doi + 1])
                        nc.sync.dma_start(
                            out_v[:, doi, sj * NT:(sj + 1) * NT], F_sb[:])

    nc.compile()
    return nc


def _get_module():
    if "nc" not in _CACHE:
        _CACHE["nc"] = _build_module()
    return _CACHE["nc"]


def _chunked(a, ncols):
    """[D, ncols] -> [P, DC, ncols] with partition p holding rows c*128+p."""
    return np.ascontiguousarray(
        a.reshape(DC, P, ncols).transpose(1, 0, 2))


def _make_in_maps(inputs):
    X = np.asarray(inputs["X"], dtype=np.float32)
    intensity = np.asarray(inputs["intensity"], dtype=np.float32)
    bf = ml_dtypes.bfloat16
    f8 = ml_dtypes.float8_e4m3
    Wq8 = _chunked(np.ascontiguousarray(
        np.asarray(inputs["Wq"], np.float32).T * 32.0), D).astype(f8)
    Wk8 = _chunked(np.ascontiguousarray(
        np.asarray(inputs["Wk"], np.float32).T * 32.0), D).astype(f8)
    WvT = _chunked(np.ascontiguousarray(
        np.asarray(inputs["Wv"], np.float32).T), D).astype(bf)
    WoT = _chunked(np.ascontiguousarray(
        np.asarray(inputs["Wo"], np.float32).T), D).astype(bf)
    bq, bk, bv, bo = (np.asarray(inputs[k], np.float32).reshape(D)
                      for k in ("bq", "bk", "bv", "bo"))
    BCOL = np.concatenate(
        [b.reshape(DC, P).T for b in (32.0 * bq, 32.0 * bk, bo)], axis=1
    ).astype(np.float32)  # [128, 24]

    in_maps = []
    for c in range(8):
        b, h = c // 2, c % 2
        XT = np.ascontiguousarray(X[b, h * SH:(h + 1) * SH, :].T)
        XQT = _chunked(XT, SH).astype(bf)
        X8T = _chunked(XT, SH).astype(f8)
        Islc = intensity[b, h * SH:(h + 1) * SH, :]
        # [t, s] -> [si*128+tp, tc*128+sp] so each per-si load is one
        # contiguous row-block (128 descriptors instead of 2048)
        IT = np.ascontiguousarray(
            Islc.T.reshape(TC, P, DC, P).transpose(2, 1, 0, 3).reshape(SH, S)
        ).astype(bf)
        rows = 1.0 + Islc.sum(axis=1, dtype=np.float64).astype(np.float32)
        BROW = np.concatenate([bv, rows]).reshape(1, D + SH)
        in_maps.append({
            "X8T": X8T, "XQT": XQT, "WQ8": Wq8, "WK8": Wk8,
            "WVT": WvT, "WOT": WoT,
            "BCOL": BCOL, "BROW": BROW, "IT": IT,
        })
    return in_maps


def _gather(results):
    out = np.empty((4, S, D), dtype=np.float32)
    for c in range(8):
        b, h = c // 2, c % 2
        out[b, h * SH:(h + 1) * SH, :] = results[c]["OUTT"].T
    return out


def kernel(**inputs):
    from concourse import bass_utils

    in_maps = _make_in_maps(inputs)
    nc = _get_module()
    res = bass_utils.run_bass_kernel_spmd(nc, in_maps, core_ids=list(range(8)))
    return _gather(res.results)
